# revision 34
# baseline (speedup 1.0000x reference)
"""AF2-style multi-head attention (B=32, S=512, HS=640, N=10, KD=64) on 8
Trainium2 NeuronCores, data-parallel over the batch dimension (4 batches/core).

Layout strategy (no on-device transposes):
  - q_data is fed transposed per batch: [a=640, tok=512] (a on partitions).
  - QKV projections produce qT/kT with head-dim on partitions and v in
    token-major layout directly.
  - Attention computes logitsT [k, q]. The pairwise bias enters as
    E = exp(nonbatched_bias)^T (host-precomputed, fp16) multiplied into
    exp(logitsT - C). The 0/1 mask bias enters as f = exp(bias) folded into
    v' = f*v plus a host-supplied f column per head, which makes the wv
    matmul emit the softmax denominator as PSUM row 64; it is broadcast via
    GPSIMD, reciprocal'd (fast NR) and multiplied in to normalize.
  - Even/odd heads use PE row groups 0-63/64-127, and their logits matmuls
    are interleaved so the PE overlaps them (row tiling).
  - The output projection contracts over head-dim on partitions and writes
    outT [o, tok]; the host transposes the result back.

All matmul inputs are fp16 (single-pass on the PE); accumulation is fp32 in
PSUM. End-to-end relative error vs the fp32 reference ~8e-4.
"""

import os

import numpy as np

B, S, HS, N, KD = 32, 512, 640, 10, 64
NCORES = 8
BL = B // NCORES          # batches per core = 4
AC = HS // 128            # hc chunks = 5
TC_ = S // 128            # token chunks = 4
CSHIFT = 4.0              # exp(logits - CSHIFT) to keep fp16 in range

_COMPILED = {}


def _build_nc():
    import concourse.bass as bass  # noqa: F401
    import concourse.mybir as mybir
    import concourse.tile as tile
    from concourse import bacc

    f32 = mybir.dt.float32
    f16 = mybir.dt.float16
    nc = bacc.Bacc("TRN2", target_bir_lowering=False, debug=False,
                   num_devices=NCORES)

    qdT = nc.dram_tensor("qdT", [BL, HS, S], f16, kind="ExternalInput").ap()
    Enb = nc.dram_tensor("Enb", [N, S, S], f16, kind="ExternalInput").ap()
    qw = nc.dram_tensor("qw", [HS, HS], f16, kind="ExternalInput").ap()
    kw = nc.dram_tensor("kw", [HS, HS], f16, kind="ExternalInput").ap()
    vw = nc.dram_tensor("vw", [HS, HS], f16, kind="ExternalInput").ap()
    ow = nc.dram_tensor("ow", [HS, HS], f16, kind="ExternalInput").ap()
    ebias = nc.dram_tensor("ebias", [128, BL * TC_], f32, kind="ExternalInput").ap()
    ob = nc.dram_tensor("ob", [128, AC], f32, kind="ExternalInput").ap()
    outT = nc.dram_tensor("outT", [BL, HS, S], f32, kind="ExternalOutput").ap()

    Exp = mybir.ActivationFunctionType.Exp

    with tile.TileContext(nc) as tc:
        import contextlib
        with contextlib.ExitStack() as ctx:
            ep = ctx.enter_context  # shorthand

            consts = ep(tc.tile_pool(name="consts", bufs=1))
            persist = ep(tc.tile_pool(name="persist", bufs=1))
            wpool = ep(tc.tile_pool(name="wpool", bufs=1))
            qdp = ep(tc.tile_pool(name="qd", bufs=1))
            attn = ep(tc.tile_pool(name="attn", bufs=1))
            osb = ep(tc.tile_pool(name="osb", bufs=3))
            # PSUM: pB (projections + outproj) 2 banks; pW (wv) 2 banks;
            # pL (logits, [128,1024] = 2 banks, bufs=2) 4 banks. Total 8.
            pB = ep(tc.tile_pool(name="pB", bufs=2, space="PSUM"))
            pw512 = ep(tc.tile_pool(name="pw512", bufs=2, space="PSUM"))
            pL = ep(tc.tile_pool(name="pL", bufs=2, space="PSUM"))

            # ---- weight + data loads (issued up front) ----
            qd_sb = [qdp.tile([128, AC, S], f16, tag=f"qd{b}", name=f"qd{b}")
                     for b in range(BL)]
            qw_sb = wpool.tile([128, AC, HS], f16)
            nc.sync.dma_start(out=qw_sb[:],
                              in_=qw.rearrange("(c p) m -> p c m", p=128))
            nc.sync.dma_start(out=qd_sb[0][:],
                              in_=qdT[0].rearrange("(c p) t -> p c t", p=128))
            kw_sb = wpool.tile([128, AC, HS], f16)
            nc.sync.dma_start(out=kw_sb[:],
                              in_=kw.rearrange("(c p) m -> p c m", p=128))
            for b in range(1, BL):
                nc.sync.dma_start(out=qd_sb[b][:],
                                  in_=qdT[b].rearrange("(c p) t -> p c t", p=128))
            vw_sb = wpool.tile([128, AC, HS], f16)
            nc.sync.dma_start(out=vw_sb[:],
                              in_=vw.rearrange("(c p) m -> p c m", p=128))
            ow_sb = wpool.tile([128, AC, HS], f16)
            nc.sync.dma_start(out=ow_sb[:],
                              in_=ow.rearrange("(c p) m -> p c m", p=128))

            ebias_sb = consts.tile([128, BL * TC_], f32)
            nc.sync.dma_start(out=ebias_sb[:], in_=ebias[:])
            ob_sb = consts.tile([128, AC], f32)
            nc.sync.dma_start(out=ob_sb[:], in_=ob[:])
            negc_sb = consts.tile([128, 1], f32)
            nc.vector.memset(negc_sb[:], -CSHIFT)
            warm_sb = consts.tile([1, 1], f32)
            nc.vector.memset(warm_sb[:], 0.0)
            nc.scalar.activation(out=warm_sb[:], in_=warm_sb[:], func=Exp,
                                 bias=negc_sb[0:1, :])

            qT_sb = [persist.tile([128, AC, S], f16, tag=f"qT{b}", name=f"qT{b}")
                     for b in range(BL)]
            kT_sb = [persist.tile([128, AC, S], f16, tag=f"kT{b}", name=f"kT{b}")
                     for b in range(BL)]
            # per head a contiguous [f*v_h | f] 65-col group
            v_sb = [[persist.tile([128, N, KD + 1], f16, tag=f"v{b}_{t}",
                                  name=f"v{b}_{t}")
                     for t in range(TC_)] for b in range(BL)]
            wvT_sb = [persist.tile([128, AC, S], f16, tag=f"wvT{b}",
                                   name=f"wvT{b}") for b in range(BL)]

            # ---------------- projection emitters ----------------
            def proj_qk_group(b, which, m):
                wsb, dst, eng = ((qw_sb, qT_sb, "act") if which == 0
                                 else (kw_sb, kT_sb, "dve"))
                ps = pB.tile([128, S], f32, tag="pb", name="ps")
                for kc in range(AC):
                    nc.tensor.matmul(
                        ps[:], wsb[:, kc, 128 * m:128 * (m + 1)],
                        qd_sb[b][:, kc, :],
                        start=(kc == 0), stop=(kc == AC - 1))
                if eng == "act":
                    nc.scalar.copy(out=dst[b][:, m, :], in_=ps[:])
                else:
                    nc.vector.tensor_copy(out=dst[b][:, m, :], in_=ps[:])

            def proj_v_group(b, t):
                pv1 = pB.tile([128, S], f32, tag="pb", name="pv1")
                pv2 = pB.tile([128, S], f32, tag="pb", name="pv2")
                for kc in range(AC):
                    lhsT = qd_sb[b][:, kc, 128 * t:128 * (t + 1)]
                    nc.tensor.matmul(pv1[:], lhsT, vw_sb[:, kc, 0:512],
                                     start=(kc == 0), stop=(kc == AC - 1))
                    nc.tensor.matmul(pv2[:, 0:128], lhsT,
                                     vw_sb[:, kc, 512:HS],
                                     start=(kc == 0), stop=(kc == AC - 1))
                vt = v_sb[b][t]
                f_col = ebias_sb[:, b * TC_ + t:b * TC_ + t + 1]
                nc.vector.tensor_scalar_mul(
                    out=vt[:, 0:8, 0:KD],
                    in0=pv1.rearrange("p (h c) -> p h c", c=KD),
                    scalar1=f_col)
                nc.vector.tensor_scalar_mul(
                    out=vt[:, 8:10, 0:KD],
                    in0=pv2[:, 0:128].rearrange("p (h c) -> p h c", c=KD),
                    scalar1=f_col)
                nc.vector.memset(vt[:, :, KD:KD + 1], 1.0)
                nc.vector.tensor_scalar_mul(
                    out=vt[:, :, KD:KD + 1], in0=vt[:, :, KD:KD + 1],
                    scalar1=f_col)

            def proj_batch_groups(b):
                return ([(proj_qk_group, (b, w, m)) for w in range(2)
                         for m in range(AC)]
                        + [(proj_v_group, (b, t)) for t in range(TC_)])

            def outproj_batch(b):
                for m in range(AC):
                    ps = pB.tile([128, S], f32, tag="pb", name="po")
                    for kc in range(AC):
                        nc.tensor.matmul(
                            ps[:], ow_sb[:, kc, 128 * m:128 * (m + 1)],
                            wvT_sb[b][:, kc, :],
                            start=(kc == 0), stop=(kc == AC - 1))
                    ot = osb.tile([128, S], f32, tag="ot", name="ot")
                    nc.vector.tensor_scalar_add(
                        out=ot[:], in0=ps[:], scalar1=ob_sb[:, m:m + 1])
                    nc.sync.dma_start(
                        out=outT[b, 128 * m:128 * (m + 1), :], in_=ot[:])

            # proj(b0) up front; the rest interleaved finely
            for fn, args in proj_batch_groups(0):
                fn(*args)
            pending = (proj_batch_groups(1) + proj_batch_groups(2)
                       + proj_batch_groups(3))

            def flush1():
                if pending:
                    fn, args = pending.pop(0)
                    fn(*args)

            # ---------------- attention: batch-major ----------------
            for b in range(BL):
                if b >= 1:
                    # this batch's projections must be complete before use
                    while len(pending) > 14 * (3 - b):
                        flush1()
                for hp in range(N // 2):
                    h0, h1 = 2 * hp, 2 * hp + 1
                    E0 = attn.tile([128, TC_, S], f16, tag="E0", bufs=3,
                                   name="E0")
                    nc.sync.dma_start(
                        out=E0[:],
                        in_=Enb[h0].rearrange("(c p) q -> p c q", p=128))
                    E1 = attn.tile([128, TC_, S], f16, tag="E1", bufs=3,
                                   name="E1")
                    nc.sync.dma_start(
                        out=E1[:],
                        in_=Enb[h1].rearrange("(c p) q -> p c q", p=128))
                    hm = hp
                    ew = [attn.tile([128, 4 * S], f16, tag=f"ew{j}", bufs=4,
                                    name=f"ew{j}") for j in range(2)]
                    for half in range(2):
                        plA = pL.tile([128, 1024], f32, tag="pl", name="plA")
                        plB = pL.tile([128, 1024], f32, tag="pl", name="plB")
                        for i in range(2):
                            kc = 2 * half + i
                            nc.tensor.matmul(
                                plA[:, 512 * i:512 * (i + 1)],
                                kT_sb[b][0:64, hm, 128 * kc:128 * (kc + 1)],
                                qT_sb[b][0:64, hm, :], start=True, stop=True)
                            nc.tensor.matmul(
                                plB[:, 512 * i:512 * (i + 1)],
                                kT_sb[b][64:128, hm, 128 * kc:128 * (kc + 1)],
                                qT_sb[b][64:128, hm, :], start=True, stop=True)
                        nc.scalar.activation(
                            out=ew[0][:, 1024 * half:1024 * (half + 1)],
                            in_=plA[:], func=Exp, bias=negc_sb[:])
                        nc.scalar.activation(
                            out=ew[1][:, 1024 * half:1024 * (half + 1)],
                            in_=plB[:], func=Exp, bias=negc_sb[:])
                    for j, (h, E) in enumerate(((h0, E0), (h1, E1))):
                        nc.vector.tensor_mul(
                            out=ew[j].rearrange("p (c q) -> p c q", q=S),
                            in0=ew[j].rearrange("p (c q) -> p c q", q=S),
                            in1=E[:])
                        pwv = pw512.tile([128, S], f32, tag="pw", name="pwv")
                        for kc in range(TC_):
                            nc.tensor.matmul(
                                pwv[0:KD + 1, :], v_sb[b][kc][:, h, :],
                                ew[j][:, 512 * kc:512 * (kc + 1)],
                                start=(kc == 0), stop=(kc == TC_ - 1))
                        flush1()   # one proj group of a later batch
                        dn = attn.tile([1, S], f32, tag="dn", bufs=3, name="dn")
                        nc.scalar.copy(out=dn[:], in_=pwv[KD:KD + 1, :])
                        rbb = attn.tile([KD, S], f32, tag="rbb", bufs=3,
                                        name="rbb")
                        nc.gpsimd.partition_broadcast(rbb[:], dn[:])
                        rb = attn.tile([KD, S], f32, tag="rb", bufs=3,
                                       name="rb")
                        nc.vector.reciprocal_approx_fast(out=rb[:], in_=rbb[:])
                        nc.vector.tensor_mul(
                            out=wvT_sb[b][64 * j:64 * j + 64, hm, :],
                            in0=pwv[0:64, :], in1=rb[:])
                    if hp >= 3:
                        flush1()
                outproj_batch(b)

    nc.compile()
    return nc


def _get_nc():
    if "nc" not in _COMPILED:
        _COMPILED["nc"] = _build_nc()
    return _COMPILED["nc"]


def _prepare_in_maps(q_data, bias, nonbatched_bias, q_w, k_w, v_w, o_w, o_b):
    q_data = np.asarray(q_data, dtype=np.float32)
    bias = np.asarray(bias, dtype=np.float32)
    nonbatched_bias = np.asarray(nonbatched_bias, dtype=np.float32)

    scale = float(KD) ** -0.5
    qw = (np.asarray(q_w, np.float32).reshape(HS, HS) * scale).astype(np.float16)
    kw = np.asarray(k_w, np.float32).reshape(HS, HS).astype(np.float16)
    vw = np.asarray(v_w, np.float32).reshape(HS, HS).astype(np.float16)
    ow = np.asarray(o_w, np.float32).reshape(HS, HS).astype(np.float16)
    ob = np.asarray(o_b, np.float32).reshape(AC, 128).T.copy()

    # E = exp(nb)^T in [h, k, q] layout, fp16
    Enb = np.exp(nonbatched_bias.transpose(0, 2, 1)).astype(np.float16)
    f = np.exp(bias.reshape(B, S)).astype(np.float32)

    in_maps = []
    for c in range(NCORES):
        sl = slice(c * BL, (c + 1) * BL)
        qdT = np.ascontiguousarray(
            q_data[sl].transpose(0, 2, 1)).astype(np.float16)
        eb = np.ascontiguousarray(
            f[sl].reshape(BL, TC_, 128).transpose(2, 0, 1).reshape(128, BL * TC_))
        in_maps.append({
            "qdT": qdT, "Enb": Enb, "qw": qw, "kw": kw, "vw": vw, "ow": ow,
            "ebias": eb, "ob": ob,
        })
    return in_maps


def kernel(q_data, bias, nonbatched_bias, q_w, k_w, v_w, o_w, o_b):
    from concourse.bass_utils import run_bass_kernel_spmd

    in_maps = _prepare_in_maps(q_data, bias, nonbatched_bias,
                               q_w, k_w, v_w, o_w, o_b)
    nc = _get_nc()
    trace = bool(os.environ.get("KERNEL_TRACE"))
    if trace:
        import shutil
        from concourse import bass_utils as _bu
        _bu.upload_artifacts = lambda d: d
        tdir = os.environ.get("KERNEL_TRACE_DIR", "/tmp/kernel_trace")
        shutil.rmtree(tdir, ignore_errors=True)
        os.makedirs(tdir, exist_ok=True)
        res = run_bass_kernel_spmd(nc, in_maps, list(range(NCORES)),
                                   trace=True, tmpdir=tdir)
        kernel.last_exec_time_ns = res.exec_time_ns
        kernel.last_results = res
    else:
        res = run_bass_kernel_spmd(nc, in_maps, list(range(NCORES)))

    out = np.empty((B, S, HS), np.float32)
    for c in range(NCORES):
        out[c * BL:(c + 1) * BL] = res.results[c]["outT"].transpose(0, 2, 1)
    return out
